# revision 4
# baseline (speedup 1.0000x reference)
"""Sliding-window (banded causal) multi-head attention on 8 TRN2 NeuronCores.

Sharding: 8 cores = 2 batches x 4 head-groups (4 heads of 64 dims each).
Each core computes QKV projections for its 4 heads, RoPE, banded flash
attention (window 1024), and a partial output projection (its 256 columns
of wo). The host sums the 4 partial outputs per batch element.

Device layout choices (per core):
  - everything bf16 on the TensorEngine, fp32 accumulation in PSUM
  - activations pre-transposed on host: xT [1024, 2048] so the in-dim is
    the contraction (partition) axis
  - Q/K produced directly transposed: QT/KT [256 outdim, 2048 tok]
  - scores computed transposed, [k, q] blocks, so probs feed the PV matmul
    as the moving operand with V as the stationary operand (no on-chip
    transposes anywhere)
  - softmax denominators come free from a ones-column appended to V
  - no running-max subtraction: logits are O(1) here, exp is safe
"""

from contextlib import ExitStack

import numpy as np
import ml_dtypes

import concourse.bass as bass
import concourse.tile as tile
from concourse import bacc, mybir
from concourse.bass_utils import run_bass_kernel_spmd

BF16 = mybir.dt.bfloat16
F32 = mybir.dt.float32
F32R = mybir.dt.float32r

B, S, H = 2, 2048, 1024
NH, HD = 16, 64
WINDOW = 1024
ROPE_THETA = 10000.0
MAX_POS = 2048
N_CORES = 8
HG = 4                      # heads per core
GD = HG * HD                # 256: head-group dim per core
P = 128
NQT = S // P                # 16 q tiles
NKT = S // P                # 16 k tiles
WT = WINDOW // P            # 8: window in tiles

_cache = {}


def _build():
    nc = bacc.Bacc("TRN2", target_bir_lowering=False, debug=False,
                   enable_asserts=False, num_devices=N_CORES)

    xT_d = nc.dram_tensor("xT", [H, S], BF16, kind="ExternalInput")
    wqT_d = nc.dram_tensor("wqT", [H, GD], BF16, kind="ExternalInput")
    wkT_d = nc.dram_tensor("wkT", [H, GD], BF16, kind="ExternalInput")
    wvT_d = nc.dram_tensor("wvT", [H, GD], BF16, kind="ExternalInput")
    woT_d = nc.dram_tensor("woT", [GD, H], BF16, kind="ExternalInput")
    cosT_d = nc.dram_tensor("cosT", [P, S], BF16, kind="ExternalInput")
    sinTs_d = nc.dram_tensor("sinTs", [P, S], BF16, kind="ExternalInput")
    bq_d = nc.dram_tensor("bq2", [P, 2], F32, kind="ExternalInput")
    bk_d = nc.dram_tensor("bk2", [P, 2], F32, kind="ExternalInput")
    maskd_d = nc.dram_tensor("maskd", [P, P], BF16, kind="ExternalInput")
    ones_d = nc.dram_tensor("ones1i", [1, HD], F32R, kind="ExternalInput")
    maskf_d = nc.dram_tensor("maskf", [P, P], BF16, kind="ExternalInput")
    out_d = nc.dram_tensor("out", [S, H], F32, kind="ExternalOutput")

    CH = H // P  # 8 contraction chunks

    with tile.TileContext(nc) as tc, ExitStack() as ctx:
        const = ctx.enter_context(tc.tile_pool(name="const", bufs=1))
        xp = ctx.enter_context(tc.tile_pool(name="xp", bufs=1))
        qk = ctx.enter_context(tc.tile_pool(name="qk", bufs=1))
        vp = ctx.enter_context(tc.tile_pool(name="vp", bufs=1))
        pp = ctx.enter_context(tc.tile_pool(name="pp", bufs=3))
        cxp = ctx.enter_context(tc.tile_pool(name="cxp", bufs=1))
        osb = ctx.enter_context(tc.tile_pool(name="osb", bufs=3))
        sm = ctx.enter_context(tc.tile_pool(name="sm", bufs=2))

        # ---- constant loads ----
        wq_t = [const.tile([P, GD], BF16, name=f"wq{c}") for c in range(CH)]
        wk_t = [const.tile([P, GD], BF16, name=f"wk{c}") for c in range(CH)]
        wv_t = [const.tile([P, GD], BF16, name=f"wv{c}") for c in range(CH)]
        wo_t = [const.tile([P, H], BF16, name=f"wo{c}") for c in range(2)]
        cosT = const.tile([P, S], BF16, name="cosT")
        sinTs = const.tile([P, S], BF16, name="sinTs")
        bq_sb = const.tile([P, 2], F32, name="bq_sb")
        bk_sb = const.tile([P, 2], F32, name="bk_sb")
        maskd = const.tile([P, P], BF16, name="maskd")
        maskf = const.tile([P, P], BF16, name="maskf")
        ones1 = const.tile([1, HD], F32R, name="ones1")

        for c in range(CH):
            nc.sync.dma_start(wq_t[c][:], wqT_d.ap()[c * P:(c + 1) * P, :])
            nc.sync.dma_start(wk_t[c][:], wkT_d.ap()[c * P:(c + 1) * P, :])
            nc.sync.dma_start(wv_t[c][:], wvT_d.ap()[c * P:(c + 1) * P, :])
        for c in range(2):
            nc.sync.dma_start(wo_t[c][:], woT_d.ap()[c * P:(c + 1) * P, :])
        nc.sync.dma_start(cosT[:], cosT_d.ap())
        nc.sync.dma_start(sinTs[:], sinTs_d.ap())
        nc.sync.dma_start(bq_sb[:], bq_d.ap())
        nc.sync.dma_start(bk_sb[:], bk_d.ap())
        nc.sync.dma_start(maskd[:], maskd_d.ap())
        nc.sync.dma_start(maskf[:], maskf_d.ap())
        nc.sync.dma_start(ones1[:], ones_d.ap())

        x_t = [xp.tile([P, S], BF16, name=f"x{c}") for c in range(CH)]
        for c in range(CH):
            nc.sync.dma_start(x_t[c][:], xT_d.ap()[c * P:(c + 1) * P, :])

        # ---- projections ----
        q_sb = [qk.tile([P, S], BF16, name=f"q{m}") for m in range(2)]
        k_sb = [qk.tile([P, S], BF16, name=f"k{m}") for m in range(2)]
        qs_sb = [qk.tile([P, S], BF16, name=f"qs{m}") for m in range(2)]
        ks_sb = [qk.tile([P, S], BF16, name=f"ks{m}") for m in range(2)]
        v_sb = [vp.tile([P, HG * (HD + 1)], BF16, name=f"v{t}")
                for t in range(NQT)]

        with tc.tile_pool(name="pj", bufs=4, space="PSUM") as pj:
            # QT / KT: out [256, 2048] as 2 partition tiles x 4 col chunks
            for w_t, dest, b_sb in ((wq_t, q_sb, bq_sb), (wk_t, k_sb, bk_sb)):
                for m in range(2):
                    for n in range(4):
                        ps = pj.tile([P, 512], F32, tag="pj", name=f"pjqk{m}{n}")
                        for c in range(CH):
                            nc.tensor.matmul(
                                ps[:], w_t[c][:, m * P:(m + 1) * P],
                                x_t[c][:, n * 512:(n + 1) * 512],
                                start=(c == 0), stop=(c == CH - 1))
                        nc.vector.tensor_scalar_add(
                            dest[m][:, n * 512:(n + 1) * 512], ps[:],
                            b_sb[:, m:m + 1])
            # V: out [2048, 256] in natural layout, strided into v_sb with
            # a ones column appended per head (softmax denominator trick)
            for t in range(NQT):
                nc.gpsimd.memset(v_sb[t][:], 1.0)
            for t in range(NQT):
                ps = pj.tile([P, GD], F32, tag="pj", name=f"pjv{t}")
                for c in range(CH):
                    nc.tensor.matmul(
                        ps[:], x_t[c][:, t * P:(t + 1) * P], wv_t[c][:],
                        start=(c == 0), stop=(c == CH - 1))
                vdst = v_sb[t].rearrange("p (h d) -> p h d", h=HG)[:, :, 0:HD]
                vsrc = ps.rearrange("p (h d) -> p h d", h=HG)
                nc.scalar.copy(vdst, vsrc)

            # ---- RoPE (in transposed layout; rotate-half = partition swap
            # of 32-rows inside each head's 64-block, via SBUF->SBUF DMA) ----
            for src, shf in ((q_sb, qs_sb), (k_sb, ks_sb)):
                for m in range(2):
                    for hb in range(2):       # two heads per tile
                        o = hb * HD
                        nc.sync.dma_start(shf[m][o:o + 32, :],
                                          src[m][o + 32:o + 64, :])
                        nc.sync.dma_start(shf[m][o + 32:o + 64, :],
                                          src[m][o:o + 32, :])
            for src, shf in ((q_sb, qs_sb), (k_sb, ks_sb)):
                for m in range(2):
                    nc.gpsimd.tensor_mul(shf[m][:], shf[m][:], sinTs[:])
                    nc.vector.tensor_mul(src[m][:], src[m][:], cosT[:])
                    nc.vector.tensor_add(src[m][:], src[m][:], shf[m][:])

        # ---- attention, one head at a time ----
        ctx_sb = [cxp.tile([P, S], BF16, name=f"cx{m}") for m in range(2)]
        with tc.tile_pool(name="sp", bufs=2, space="PSUM") as sp, \
             tc.tile_pool(name="cp", bufs=1, space="PSUM") as cp:
            for h in range(HG):
                mt, ho = h // 2, (h % 2) * HD
                qh = q_sb[mt][ho:ho + HD, :]
                kh = k_sb[mt][ho:ho + HD, :]
                for qg in range(NQT // 4):
                    ctx_ps = cp.tile([HD + 1, 512], F32, tag="ctx",
                                     name=f"ctxps{h}{qg}")
                    for qj in range(4):
                        qi = qg * 4 + qj
                        kt0 = max(0, qi - WT)
                        nkt = qi - kt0 + 1
                        # scores^T blocks [k,q], stacked along free axis
                        s_ps = sp.tile([P, WT * P + P], F32, tag="sp",
                                       name=f"sps{h}{qi}")
                        for i, kt in enumerate(range(kt0, qi + 1)):
                            nc.tensor.matmul(
                                s_ps[:, i * P:(i + 1) * P],
                                kh[:, kt * P:(kt + 1) * P],
                                qh[:, qi * P:(qi + 1) * P],
                                start=True, stop=True)
                        probs = pp.tile([P, WT * P + P], BF16, tag="pp",
                                        name=f"pr{h}{qi}")
                        nc.scalar.activation(
                            probs[:, 0:nkt * P], s_ps[:, 0:nkt * P],
                            mybir.ActivationFunctionType.Exp,
                            scale=float(1.0 / np.sqrt(HD)))
                        # band-edge masks (multiplicative, post-exp)
                        nc.vector.tensor_mul(
                            probs[:, (nkt - 1) * P:nkt * P],
                            probs[:, (nkt - 1) * P:nkt * P], maskd[:])
                        if qi >= WT:
                            nc.vector.tensor_mul(
                                probs[:, 0:P], probs[:, 0:P], maskf[:])
                        # ctx^T [65, q]: stationary V (with ones col)
                        for i, kt in enumerate(range(kt0, qi + 1)):
                            nc.tensor.matmul(
                                ctx_ps[:, qj * P:(qj + 1) * P],
                                v_sb[kt][:, h * (HD + 1):(h + 1) * (HD + 1)],
                                probs[:, i * P:(i + 1) * P],
                                start=(i == 0), stop=(i == nkt - 1))
                    # normalize 4 q-tiles at once: rows 0:64 / row 64
                    rinv = sm.tile([1, 512], F32R, tag="rinv", name=f"ri{h}{qg}")
                    with nc.allow_low_precision(
                            reason="f32r view; storage is full fp32 bits"):
                        nc.vector.reciprocal(rinv[:], ctx_ps[HD:HD + 1, :])
                    rbc = cp.tile([HD, 512], F32, tag="rbc", name=f"rb{h}{qg}")
                    nc.tensor.matmul(rbc[:], ones1[:], rinv[:],
                                     start=True, stop=True)
                    dst = ctx_sb[mt][ho:ho + HD, qg * 512:(qg + 1) * 512]
                    nc.vector.tensor_copy(dst, ctx_ps[0:HD, :])
                    nc.vector.tensor_mul(dst, dst, rbc[:])

        # ---- output projection: out[t, o] = ctx^T.T @ woT ----
        with tc.tile_pool(name="po", bufs=4, space="PSUM") as po:
            for t in range(NQT):
                o_sb = osb.tile([P, H], F32, tag="osb", name=f"ot{t}")
                for n in range(2):
                    ps = po.tile([P, 512], F32, tag="po", name=f"pso{t}{n}")
                    for c in range(2):
                        nc.tensor.matmul(
                            ps[:], ctx_sb[c][:, t * P:(t + 1) * P],
                            wo_t[c][:, n * 512:(n + 1) * 512],
                            start=(c == 0), stop=(c == 1))
                    nc.scalar.copy(o_sb[:, n * 512:(n + 1) * 512], ps[:])
                nc.sync.dma_start(out_d.ap()[t * P:(t + 1) * P, :], o_sb[:])

    nc.compile()
    return nc


def _rope_tables():
    inv_freq = 1.0 / (ROPE_THETA ** (np.arange(0, HD, 2, dtype=np.float64) / HD))
    t = np.arange(MAX_POS, dtype=np.float64)
    freqs = np.outer(t, inv_freq)                       # [MAX_POS, 32]
    emb = np.concatenate([freqs, freqs], axis=-1)       # [MAX_POS, 64]
    return np.cos(emb).astype(np.float32), np.sin(emb).astype(np.float32)


def kernel(hidden_states, position_ids, wq, bq, wk, bk, wv, bv, wo, bo):
    bf16 = ml_dtypes.bfloat16
    if "nc" not in _cache:
        _cache["nc"] = _build()
    nc = _cache["nc"]

    cos_t, sin_t = _rope_tables()
    pos = np.clip(np.asarray(position_ids), 0, MAX_POS - 1).astype(np.int64)

    maskd = np.triu(np.ones((P, P), np.float32)).astype(bf16)       # k <= q
    maskf = np.tril(np.ones((P, P), np.float32), -1).astype(bf16)   # k > q

    in_maps = []
    for core in range(N_CORES):
        b, g = core // HG, core % HG
        sl = slice(g * GD, (g + 1) * GD)
        cos_b = cos_t[pos[b]]                            # [S, 64]
        sin_b = sin_t[pos[b]]
        cosT = np.tile(cos_b.T, (2, 1)).astype(bf16)     # [128, S]
        sin_sgn = sin_b.T.copy()                         # [64, S]
        sin_sgn[0:32] *= -1.0
        sinTs = np.tile(sin_sgn, (2, 1)).astype(bf16)
        in_maps.append({
            "xT": np.ascontiguousarray(hidden_states[b].T).astype(bf16),
            "wqT": np.ascontiguousarray(wq[sl].T).astype(bf16),
            "wkT": np.ascontiguousarray(wk[sl].T).astype(bf16),
            "wvT": np.ascontiguousarray(wv[sl].T).astype(bf16),
            "woT": np.ascontiguousarray(wo[:, sl].T).astype(bf16),
            "cosT": cosT,
            "sinTs": sinTs,
            "bq2": np.ascontiguousarray(
                bq[sl].reshape(2, P).T).astype(np.float32),
            "bk2": np.ascontiguousarray(
                bk[sl].reshape(2, P).T).astype(np.float32),
            "maskd": maskd,
            "ones1i": np.ones((1, HD), np.float32),
            "maskf": maskf,
        })

    res = run_bass_kernel_spmd(nc, in_maps, core_ids=list(range(N_CORES)))

    const_off = (wo @ bv + bo).astype(np.float32)        # host-folded biases
    out = np.empty((B, S, H), dtype=np.float32)
    for b in range(B):
        acc = res.results[b * HG]["out"].astype(np.float32).copy()
        for g in range(1, HG):
            acc += res.results[b * HG + g]["out"]
        out[b] = acc + const_off[None, :]
    return out
